# revision 24
# baseline (speedup 1.0000x reference)
"""Trainium2 Bass kernel for temporal-window GNN mean aggregation.

    out = x + scatter_mean(x[src] * mask, dst),
    mask = (edge_time <= seed_time[dst]) & (edge_time > seed_time[dst] - 100)

Two-pass design; all reference arithmetic happens on device, host work is
layout only (permutation / compaction / padding by device-computed values).

Pass 1 (device): DVE computes the temporal mask for all E edges from
  metadata planes (edge_time, seed_time[dst]) laid out by the host.
Host: reads the mask back and uses it purely for layout: keeps only masked
  edges (~10%), packs the active destination nodes (cnt>=1) into balanced
  256-dst "window pairs", shards pairs across the 8 cores, and builds
  int16 gather-index planes + one-hot metadata for pass 2.  Inactive
  destinations get out = x + 0 = x, copied by the host.
Pass 2 (device): per core, 4 banked SWDGE dma_gathers fetch the masked
  x16 rows (512B = 128 fp16 features + 1.0 ones column); DVE builds the
  one-hot S from dst metadata; PE accumulates PSUM[dst 0:129] via
  S^T @ G matmuls (2 windows per 128-slot block, 4 banks); counts ride
  the ones column; ACT applies 1/max(cnt,1); DVE adds the residual.
"""

import math
import sys

import numpy as np

for _p in ("/opt/trn_rl_repo",):
    if _p not in sys.path:
        sys.path.insert(0, _p)

import concourse.bass as bass
import concourse.mybir as mybir
import concourse.tile as tile
from concourse import bacc
from concourse.bass_utils import run_bass_kernel_spmd

P = 128            # SBUF partitions == dst-window size == slot-block size
D = 128            # feature dim
NCORES = 8
N = 100000
E = 600000
TIME_WINDOW = 100

NBANKS = 4         # int16 gather-index banks over x16 rows
BANK = 25089       # rows per bank (<= 32768), NBANKS*BANK >= N
XROWS = NBANKS * BANK

# pass-1 edge grid
CE = 586                       # cols per partition; 128*CE = 75008 >= E/8
EPC = P * CE                   # edges per core (padded)
EPAD = NCORES * EPC

f32 = mybir.dt.float32
f16 = mybir.dt.float16
i32 = mybir.dt.int32
i16 = mybir.dt.int16
OP = mybir.AluOpType


# ---------------------------------------------------------------- pass 1
def build_program1():
    nc = bacc.Bacc("TRN2", target_bir_lowering=False, debug=False,
                   num_devices=NCORES)
    etp = nc.dram_tensor("etp", [P, CE], f16, kind="ExternalInput")
    stp = nc.dram_tensor("stp", [P, CE], f16, kind="ExternalInput")
    msk = nc.dram_tensor("msk", [P, CE], f16, kind="ExternalOutput")
    with tile.TileContext(nc) as tc:
        with tc.tile_pool(name="m", bufs=1) as pool:
            et_t = pool.tile([P, CE], f16)
            st_t = pool.tile([P, CE], f16)
            nc.sync.dma_start(out=et_t[:], in_=etp[:])
            nc.sync.dma_start(out=st_t[:], in_=stp[:])
            d_t = pool.tile([P, CE], f16)
            nc.vector.tensor_tensor(out=d_t[:], in0=st_t[:], in1=et_t[:],
                                    op=OP.subtract)
            m1_t = pool.tile([P, CE], f16)
            nc.vector.tensor_scalar(out=m1_t[:], in0=d_t[:], scalar1=0.0,
                                    scalar2=None, op0=OP.is_ge)
            m2_t = pool.tile([P, CE], f16)
            nc.vector.tensor_scalar(out=m2_t[:], in0=d_t[:],
                                    scalar1=float(TIME_WINDOW),
                                    scalar2=None, op0=OP.is_lt)
            m_t = pool.tile([P, CE], f16)
            nc.vector.tensor_tensor(out=m_t[:], in0=m1_t[:], in1=m2_t[:],
                                    op=OP.mult)
            nc.sync.dma_start(out=msk[:], in_=m_t[:])
    nc.compile()
    return nc


# ---------------------------------------------------------------- pass 2
def build_program2(PC: int, CPs: tuple):
    """PC pairs of 128-dst windows per core, chunked into CPs pairs/chunk.

    Per (chunk, bank): one dma_gather of CP*128 slots; block b holds pair
    pk's bank-j masked edges (<=128 by host packing).  Each block feeds two
    matmuls (window contexts 2*pk and 2*pk+1) via iota2-vs-dl one-hot.
    """
    assert sum(CPs) == PC
    CPmax = max(CPs)
    nc = bacc.Bacc(
        "TRN2", target_bir_lowering=False, debug=False, num_devices=NCORES,
        num_swdge_queues=4,
    )

    TOTI = PC * NBANKS * P // 16          # total idx cols (int16, wrapped)
    x16 = nc.dram_tensor("x16", [XROWS, 2 * D], f16, kind="ExternalInput")
    xs = nc.dram_tensor("xs", [PC * 2 * P, D], f32, kind="ExternalInput")
    idx16 = nc.dram_tensor("idx16", [P, TOTI], i16, kind="ExternalInput")
    dlp = nc.dram_tensor("dlp", [P, NBANKS * PC], f16, kind="ExternalInput")
    out = nc.dram_tensor("out", [PC * 2 * P, D], f32, kind="ExternalOutput")

    with tile.TileContext(nc) as tc:
        with (
            tc.tile_pool(name="meta", bufs=1) as meta,
            tc.tile_pool(name="oc", bufs=2) as oc,
            tc.tile_pool(name="small", bufs=4) as small,
            tc.tile_pool(name="psum", bufs=4, space="PSUM") as psum_tp,
        ):
            # lean preamble: ONLY the idx load gates the gathers.  dl/iota/
            # xs loads are needed no earlier than the S-build / finalize and
            # are issued after chunk-0's gathers are on the queue.
            dl_t = meta.tile([P, NBANKS * PC], f16)
            idx_t = meta.tile([P, TOTI], i16)
            nc.sync.dma_start(out=idx_t[:], in_=idx16[:])

            iota_i = meta.tile([P, 2 * P], i32)
            iota_f = meta.tile([P, 2 * P], f16)
            x_ts = [
                meta.tile([P, 2 * CP * D], f32, name=f"x_{c}")
                for c, CP in enumerate(CPs)
            ]

            ioff = 0
            boff = 0
            roff = 0
            for c, CP in enumerate(CPs):
                NIDX = CP * P
                ICOLS = NIDX // 16
                g_ts = []
                for j in range(NBANKS):
                    g_t = meta.tile([P, CP * 2 * D], f16,
                                    name=f"g_{c}_{j}")
                    nc.gpsimd.dma_gather(
                        out_ap=g_t[:].rearrange("p (k c) -> p k c", c=2 * D),
                        in_ap=x16[j * BANK :, :],
                        idxs_ap=idx_t[:, ioff + j * ICOLS
                                      : ioff + (j + 1) * ICOLS],
                        num_idxs=NIDX,
                        num_idxs_reg=NIDX,
                        elem_size=2 * D,
                        single_packet=False,
                        queue_num=j,
                    )
                    g_ts.append(g_t)
                ioff += NBANKS * ICOLS

                if c == 0:
                    # deferred loads: dl for the S-builds, iota ramp, and
                    # the hoisted residual-x streams for every chunk.
                    nc.sync.dma_start(out=dl_t[:], in_=dlp[:])
                    nc.gpsimd.iota(iota_i[:], pattern=[[0, 1], [1, 2 * P]],
                                   base=0, channel_multiplier=0)
                    nc.vector.tensor_copy(out=iota_f[:], in_=iota_i[:])
                    roff0 = 0
                    for cc, CPc in enumerate(CPs):
                        RPCc = 2 * CPc * P
                        nc.sync.dma_start(
                            out=x_ts[cc][:],
                            in_=xs[roff0 : roff0 + RPCc, :].rearrange(
                                "(p w) d -> p (w d)", p=P
                            ),
                        )
                        roff0 += RPCc

                BC = NBANKS * CP
                s_t = meta.tile([P, BC * 2 * P], f16, name=f"s_{c}")
                nc.vector.tensor_tensor(
                    out=s_t[:].rearrange("p (b m) -> p b m", m=2 * P),
                    in0=iota_f[:].unsqueeze(1).to_broadcast([P, BC, 2 * P]),
                    in1=dl_t[:, boff : boff + BC]
                    .unsqueeze(2)
                    .to_broadcast([P, BC, 2 * P]),
                    op=OP.is_equal,
                )
                boff += BC

                RPC = 2 * CP * P
                x_t = x_ts[c]

                o_t = oc.tile([P, 2 * CPmax * D], f32, tag="o")
                for pk in range(CP):
                    ps = [psum_tp.tile([P, D + 1], f32, tag=f"ps{ctx}",
                                       name=f"ps{ctx}_{c}_{pk}")
                          for ctx in range(2)]
                    for j in range(NBANKS):
                        b = j * CP + pk
                        for ctx in range(2):
                            nc.tensor.matmul(
                                out=ps[ctx][:],
                                lhsT=s_t[:, b * 2 * P + ctx * P
                                         : b * 2 * P + (ctx + 1) * P],
                                rhs=g_ts[j][:, pk * 2 * D : pk * 2 * D + D + 1],
                                start=(j == 0),
                                stop=(j == NBANKS - 1),
                            )
                    for ctx in range(2):
                        wloc = 2 * pk + ctx
                        cnt_t = small.tile([P, 1], f32, tag="cnt")
                        nc.vector.tensor_scalar(out=cnt_t[:],
                                                in0=ps[ctx][:, D : D + 1],
                                                scalar1=1.0, scalar2=None,
                                                op0=OP.max)
                        rcp_t = small.tile([P, 1], f32, tag="rcp")
                        nc.vector.reciprocal(out=rcp_t[:], in_=cnt_t[:])
                        osl = o_t[:, wloc * D : (wloc + 1) * D]
                        nc.scalar.activation(
                            out=osl,
                            in_=ps[ctx][:, 0:D],
                            func=mybir.ActivationFunctionType.Copy,
                            scale=rcp_t[:, 0:1],
                        )
                        nc.vector.tensor_tensor(
                            out=osl, in0=osl,
                            in1=x_t[:, wloc * D : (wloc + 1) * D],
                            op=OP.add,
                        )

                nc.sync.dma_start(
                    out=out[roff : roff + RPC, :].rearrange(
                        "(p w) d -> p (w d)", p=P
                    ),
                    in_=o_t[:, : 2 * CP * D],
                )
                roff += RPC

    nc.compile()
    return nc


_P1_CACHE: list = []
_P2_CACHE: dict = {}


def _get_program1():
    if not _P1_CACHE:
        _P1_CACHE.append(build_program1())
    return _P1_CACHE[0]


def _get_program2(PC, CPs):
    key = (PC, tuple(CPs))
    if key not in _P2_CACHE:
        _P2_CACHE[key] = build_program2(PC, tuple(CPs))
    return _P2_CACHE[key]


# ---------------------------------------------------------------- host prep
def _prep_pass1(edge_time, seed_time, dst):
    et = np.asarray(edge_time).astype(np.float16)
    st_d = np.asarray(seed_time)[dst].astype(np.float16)
    etp = np.full(EPAD, 30000.0, dtype=np.float16)
    stp = np.zeros(EPAD, dtype=np.float16)
    etp[:E] = et
    stp[:E] = st_d
    etp = etp.reshape(NCORES, P, CE)
    stp = stp.reshape(NCORES, P, CE)
    return [{"etp": etp[c], "stp": stp[c]} for c in range(NCORES)]


def _decode_mask(res):
    m = np.stack([res.results[c]["msk"] for c in range(NCORES)])
    return m.reshape(-1)[:E] > 0.5


def _pack_pairs(cnt_db, active):
    """Snake-deal active dsts (sorted by total cnt desc) into window pairs,
    then fill to 256 dsts/pair with inactive fillers.  Returns
    (PC, pair_dsts [8*PC, 256] int64)."""
    tot = cnt_db.sum(1)
    act_ids = np.flatnonzero(active)
    A = act_ids.size
    PC = max(1, math.ceil(A / (NCORES * 256)))
    while True:
        npairs = NCORES * PC
        order = act_ids[np.argsort(-tot[act_ids], kind="stable")]
        # snake deal
        cols = np.arange(order.size)
        rnd = cols // npairs
        pos = cols % npairs
        pair_of = np.where(rnd % 2 == 0, pos, npairs - 1 - pos)
        # per-pair per-bank loads
        loads = np.zeros((npairs, NBANKS), dtype=np.int64)
        np.add.at(loads, pair_of, cnt_db[order])
        counts = np.bincount(pair_of, minlength=npairs)
        if loads.max() <= P and counts.max() <= 256:
            break
        PC += 1
    # fill with inactive dsts
    pair_dsts = np.full((npairs, 256), -1, dtype=np.int64)
    perm = np.argsort(pair_of, kind="stable")
    sor = order[perm]
    po = pair_of[perm]
    # position within pair
    first = np.r_[True, po[1:] != po[:-1]]
    idx0 = np.flatnonzero(first)
    runlen = np.diff(np.r_[idx0, po.size])
    within = np.arange(po.size) - np.repeat(idx0, runlen)
    pair_dsts[po, within] = sor
    # fillers: inactive dsts
    inact = np.flatnonzero(~active)
    need = (pair_dsts < 0)
    nneed = need.sum()
    fill = np.full(nneed, -1, dtype=np.int64)
    take = min(nneed, inact.size)
    fill[:take] = inact[:take]
    pair_dsts[need] = fill
    used_inact = inact[:take]
    return PC, pair_dsts, used_inact


def _prep_pass2(x, src, dst, mask):
    x = np.asarray(x, dtype=np.float32)
    me = np.flatnonzero(mask)
    src_m = src[me]
    dst_m = dst[me]
    bank_m = src_m // BANK
    cnt_db = np.zeros((N, NBANKS), dtype=np.int64)
    np.add.at(cnt_db, (dst_m, bank_m), 1)
    active = cnt_db.sum(1) > 0
    PC, pair_dsts, used_inact = _pack_pairs(cnt_db, active)
    npairs = NCORES * PC
    NCH = min(2, PC)
    base = PC // NCH
    rem = PC % NCH
    CPs = tuple(base + (1 if i < rem else 0) for i in range(NCH))

    # dst -> (core-local slot) mapping
    dloc = np.full(N, -1, dtype=np.int64)       # core-local dst index
    dcore = np.full(N, -1, dtype=np.int64)
    pd = pair_dsts.reshape(NCORES, PC, 256)
    for c in range(NCORES):
        ids = pd[c]
        v = ids >= 0
        dloc[ids[v]] = (np.arange(PC * 256).reshape(PC, 256))[v]
        dcore[ids[v]] = c

    # masked edge placement: (core, pair-local pc, bank, slot)
    ecore = dcore[dst_m]
    epair = dloc[dst_m] // 256                  # pair index within core
    # rank within (core, pair, bank)
    key = ((ecore * PC + epair) * NBANKS + bank_m).astype(np.int64)
    eorder = np.argsort(key, kind="stable")
    ks = key[eorder]
    binc = np.bincount(key, minlength=NCORES * PC * NBANKS)
    assert binc.max() <= P
    offs = np.zeros(NCORES * PC * NBANKS, dtype=np.int64)
    np.cumsum(binc[:-1], out=offs[1:])
    rank = np.arange(me.size, dtype=np.int64) - offs[ks]

    cso = ecore[eorder]
    pso = epair[eorder]
    jso = bank_m[eorder]
    srcso = src_m[eorder]
    dlocso = dloc[dst_m][eorder]

    # chunk bookkeeping
    chunk_of_pair = np.zeros(PC, dtype=np.int64)
    pk_in_chunk = np.zeros(PC, dtype=np.int64)
    cstart = 0
    ioffs = []      # idx col offset per (chunk)
    boffs = []
    ioff = 0
    boff = 0
    for ci, CP in enumerate(CPs):
        chunk_of_pair[cstart : cstart + CP] = ci
        pk_in_chunk[cstart : cstart + CP] = np.arange(CP)
        ioffs.append(ioff)
        boffs.append(boff)
        ioff += NBANKS * CP * P // 16
        boff += NBANKS * CP
        cstart += CP
    TOTI = ioff

    co = chunk_of_pair[pso]
    pko = pk_in_chunk[pso]
    CPs_a = np.asarray(CPs)
    ioffs_a = np.asarray(ioffs)
    boffs_a = np.asarray(boffs)

    # gather idx planes: call (chunk, bank j): position i = pk*128 + rank
    pos = pko * P + rank
    icol = ioffs_a[co] + jso * (CPs_a[co] * P // 16) + pos // 16
    irow = pos % 16
    idx_a = np.zeros((NCORES, 16, TOTI), dtype=np.int16)
    idx_a[cso, irow, icol] = (srcso - jso * BANK).astype(np.int16)
    idx_rep = np.tile(idx_a, (1, 8, 1))

    # dl plane: col = boff_c + j*CP + pk ; value = dloc - pc*256
    dl_a = np.full((NCORES, P, NBANKS * PC), 30000.0, dtype=np.float16)
    dcol = boffs_a[co] + jso * CPs_a[co] + pko
    dl_a[cso, rank, dcol] = (dlocso - pso * 256).astype(np.float16)

    # xs rows, permuted per chunk: row = roff_c + p*(2*CP) + wloc
    xg = np.zeros((NCORES, PC * 256, D), dtype=np.float32)
    for c in range(NCORES):
        ids = pd[c].reshape(-1)
        v = ids >= 0
        xg[c][v] = x[ids[v]]
    xs_l = []
    for c in range(NCORES):
        rows = []
        rbase = 0
        for ci, CP in enumerate(CPs):
            blk = xg[c][rbase : rbase + CP * 256]     # (wloc p) d  w/ dloc = wloc*128+p
            blk = blk.reshape(2 * CP, P, D).transpose(1, 0, 2).reshape(-1, D)
            rows.append(blk)
            rbase += CP * 256
        xs_l.append(np.ascontiguousarray(np.concatenate(rows, 0)))

    x16 = np.zeros((XROWS, 2 * D), dtype=np.float16)
    x16[:N, :D] = x.astype(np.float16)
    x16[:, D] = 1.0

    in_maps = [
        {
            "x16": x16,
            "xs": xs_l[c],
            "idx16": idx_rep[c],
            "dlp": dl_a[c],
        }
        for c in range(NCORES)
    ]
    meta = {"PC": PC, "CPs": CPs, "pd": pd, "used_inact": used_inact}
    return in_maps, meta


def _assemble(x, res, meta):
    PC, CPs, pd = meta["PC"], meta["CPs"], meta["pd"]
    out = np.asarray(x, dtype=np.float32).copy()
    for c in range(NCORES):
        o = res.results[c]["out"]
        rbase = 0
        rows = []
        for CP in CPs:
            blk = o[rbase : rbase + CP * 256]
            blk = blk.reshape(P, 2 * CP, D).transpose(1, 0, 2).reshape(-1, D)
            rows.append(blk)
            rbase += CP * 256
        o_un = np.concatenate(rows, 0)           # dloc-ordered
        ids = pd[c].reshape(-1)
        v = ids >= 0
        out[ids[v]] = o_un[v]
    return out


def kernel(x, edge_index, edge_time, seed_time):
    ei = np.asarray(edge_index)
    src = ei[0].astype(np.int64)
    dst = ei[1].astype(np.int64)

    p1_maps = _prep_pass1(edge_time, seed_time, dst)
    nc1 = _get_program1()
    res1 = run_bass_kernel_spmd(nc1, p1_maps, core_ids=list(range(NCORES)))
    mask = _decode_mask(res1)

    p2_maps, meta = _prep_pass2(x, src, dst, mask)
    nc2 = _get_program2(meta["PC"], meta["CPs"])
    res2 = run_bass_kernel_spmd(nc2, p2_maps, core_ids=list(range(NCORES)))
    return _assemble(x, res2, meta).astype(np.float32)


# revision 26
# speedup vs baseline: 1.2355x; 1.2355x over previous
"""Trainium2 Bass kernel for temporal-window GNN mean aggregation.

    out = x + scatter_mean(x[src] * mask, dst),
    mask = (edge_time <= seed_time[dst]) & (edge_time > seed_time[dst] - 100)

Two-pass design; all reference arithmetic happens on device, host work is
layout only (permutation / compaction / padding by device-computed values).

Pass 1 (device): DVE computes the temporal mask for all E edges from
  metadata planes (edge_time, seed_time[dst]) laid out by the host.
Host: reads the mask back and uses it purely for layout: keeps only masked
  edges (~10%), packs the active destination nodes (cnt>=1) into balanced
  256-dst "window pairs", shards pairs across the 8 cores, and builds
  int16 gather-index planes + one-hot metadata for pass 2.  Inactive
  destinations get out = x + 0 = x, copied by the host.
Pass 2 (device): per core, 4 banked SWDGE dma_gathers fetch the masked
  x16 rows (512B = 128 fp16 features + 1.0 ones column); DVE builds the
  one-hot S from dst metadata; PE accumulates PSUM[dst 0:129] via
  S^T @ G matmuls (2 windows per 128-slot block, 4 banks); counts ride
  the ones column; ACT applies 1/max(cnt,1); DVE adds the residual.
"""

import math
import sys

import numpy as np

for _p in ("/opt/trn_rl_repo",):
    if _p not in sys.path:
        sys.path.insert(0, _p)

import concourse.bass as bass
import concourse.mybir as mybir
import concourse.tile as tile
from concourse import bacc
from concourse.bass_utils import run_bass_kernel_spmd

P = 128            # SBUF partitions == dst-window size == slot-block size
D = 128            # feature dim
NCORES = 8
N = 100000
E = 600000
TIME_WINDOW = 100

NBANKS = 4         # int16 gather-index banks over x16 rows
BANK = 25089       # rows per bank (<= 32768), NBANKS*BANK >= N
XROWS = NBANKS * BANK

# pass-1 edge grid
CE = 586                       # cols per partition; 128*CE = 75008 >= E/8
EPC = P * CE                   # edges per core (padded)
EPAD = NCORES * EPC

f32 = mybir.dt.float32
f16 = mybir.dt.float16
i32 = mybir.dt.int32
i16 = mybir.dt.int16
OP = mybir.AluOpType


# ---------------------------------------------------------------- pass 1
def build_program1():
    nc = bacc.Bacc("TRN2", target_bir_lowering=False, debug=False,
                   num_devices=NCORES)
    etp = nc.dram_tensor("etp", [P, CE], f16, kind="ExternalInput")
    stp = nc.dram_tensor("stp", [P, CE], f16, kind="ExternalInput")
    msk = nc.dram_tensor("msk", [P, CE], f16, kind="ExternalOutput")
    with tile.TileContext(nc) as tc:
        with tc.tile_pool(name="m", bufs=1) as pool:
            et_t = pool.tile([P, CE], f16)
            st_t = pool.tile([P, CE], f16)
            nc.sync.dma_start(out=et_t[:], in_=etp[:])
            nc.sync.dma_start(out=st_t[:], in_=stp[:])
            d_t = pool.tile([P, CE], f16)
            nc.vector.tensor_tensor(out=d_t[:], in0=st_t[:], in1=et_t[:],
                                    op=OP.subtract)
            m1_t = pool.tile([P, CE], f16)
            nc.vector.tensor_scalar(out=m1_t[:], in0=d_t[:], scalar1=0.0,
                                    scalar2=None, op0=OP.is_ge)
            m2_t = pool.tile([P, CE], f16)
            nc.vector.tensor_scalar(out=m2_t[:], in0=d_t[:],
                                    scalar1=float(TIME_WINDOW),
                                    scalar2=None, op0=OP.is_lt)
            m_t = pool.tile([P, CE], f16)
            nc.vector.tensor_tensor(out=m_t[:], in0=m1_t[:], in1=m2_t[:],
                                    op=OP.mult)
            nc.sync.dma_start(out=msk[:], in_=m_t[:])
    nc.compile()
    return nc


# ---------------------------------------------------------------- pass 2
def build_program2(PC: int, CPs: tuple):
    """PC pairs of 128-dst windows per core, chunked into CPs pairs/chunk.

    Per (chunk, bank): one dma_gather of CP*128 slots; block b holds pair
    pk's bank-j masked edges (<=128 by host packing).  Each block feeds two
    matmuls (window contexts 2*pk and 2*pk+1) via iota2-vs-dl one-hot.
    """
    assert sum(CPs) == PC
    CPmax = max(CPs)
    nc = bacc.Bacc(
        "TRN2", target_bir_lowering=False, debug=False, num_devices=NCORES,
        num_swdge_queues=4,
    )

    TOTI = PC * NBANKS * P // 16          # total idx cols (int16, wrapped)
    x16 = nc.dram_tensor("x16", [XROWS, 2 * D], f16, kind="ExternalInput")
    xs = nc.dram_tensor("xs", [PC * 2 * P, D], f32, kind="ExternalInput")
    idx16 = nc.dram_tensor("idx16", [P, TOTI], i16, kind="ExternalInput")
    dlp = nc.dram_tensor("dlp", [P, NBANKS * PC], f16, kind="ExternalInput")
    out = nc.dram_tensor("out", [PC * 2 * P, D], f32, kind="ExternalOutput")

    with tile.TileContext(nc) as tc:
        with (
            tc.tile_pool(name="meta", bufs=1) as meta,
            tc.tile_pool(name="oc", bufs=2) as oc,
            tc.tile_pool(name="small", bufs=4) as small,
            tc.tile_pool(name="psum", bufs=4, space="PSUM") as psum_tp,
        ):
            # lean preamble: ONLY the idx load gates the gathers.  dl/iota/
            # xs loads are needed no earlier than the S-build / finalize and
            # are issued after chunk-0's gathers are on the queue.
            dl_t = meta.tile([P, NBANKS * PC], f16)
            idx_t = meta.tile([P, TOTI], i16)
            nc.sync.dma_start(out=idx_t[:], in_=idx16[:])

            iota_i = meta.tile([P, 2 * P], i32)
            iota_f = meta.tile([P, 2 * P], f16)

            # hoisted residual-x reads: issued up front so this HWDGE
            # traffic drains before the gather transfers stream.
            x_ts = []
            roff0 = 0
            for c, CP in enumerate(CPs):
                RPC0 = 2 * CP * P
                x_t = meta.tile([P, 2 * CP * D], f32, name=f"x_{c}")
                nc.sync.dma_start(
                    out=x_t[:],
                    in_=xs[roff0 : roff0 + RPC0, :].rearrange(
                        "(p w) d -> p (w d)", p=P
                    ),
                )
                x_ts.append(x_t)
                roff0 += RPC0

            ioff = 0
            boff = 0
            roff = 0
            for c, CP in enumerate(CPs):
                NIDX = CP * P
                ICOLS = NIDX // 16
                g_ts = []
                for j in range(NBANKS):
                    g_t = meta.tile([P, CP * 2 * D], f16,
                                    name=f"g_{c}_{j}")
                    nc.gpsimd.dma_gather(
                        out_ap=g_t[:].rearrange("p (k c) -> p k c", c=2 * D),
                        in_ap=x16[j * BANK :, :],
                        idxs_ap=idx_t[:, ioff + j * ICOLS
                                      : ioff + (j + 1) * ICOLS],
                        num_idxs=NIDX,
                        num_idxs_reg=NIDX,
                        elem_size=2 * D,
                        single_packet=False,
                        queue_num=j,
                    )
                    g_ts.append(g_t)
                ioff += NBANKS * ICOLS

                if c == 0:
                    # deferred: dl (S-build input) and the iota ramp are not
                    # needed before the matmul phase.
                    nc.sync.dma_start(out=dl_t[:], in_=dlp[:])
                    nc.gpsimd.iota(iota_i[:], pattern=[[0, 1], [1, 2 * P]],
                                   base=0, channel_multiplier=0)
                    nc.vector.tensor_copy(out=iota_f[:], in_=iota_i[:])

                BC = NBANKS * CP
                s_t = meta.tile([P, BC * 2 * P], f16, name=f"s_{c}")
                nc.vector.tensor_tensor(
                    out=s_t[:].rearrange("p (b m) -> p b m", m=2 * P),
                    in0=iota_f[:].unsqueeze(1).to_broadcast([P, BC, 2 * P]),
                    in1=dl_t[:, boff : boff + BC]
                    .unsqueeze(2)
                    .to_broadcast([P, BC, 2 * P]),
                    op=OP.is_equal,
                )
                boff += BC

                RPC = 2 * CP * P
                x_t = x_ts[c]

                o_t = oc.tile([P, 2 * CPmax * D], f32, tag="o")
                for pk in range(CP):
                    ps = [psum_tp.tile([P, D + 1], f32, tag=f"ps{ctx}",
                                       name=f"ps{ctx}_{c}_{pk}")
                          for ctx in range(2)]
                    for j in range(NBANKS):
                        b = j * CP + pk
                        for ctx in range(2):
                            nc.tensor.matmul(
                                out=ps[ctx][:],
                                lhsT=s_t[:, b * 2 * P + ctx * P
                                         : b * 2 * P + (ctx + 1) * P],
                                rhs=g_ts[j][:, pk * 2 * D : pk * 2 * D + D + 1],
                                start=(j == 0),
                                stop=(j == NBANKS - 1),
                            )
                    for ctx in range(2):
                        wloc = 2 * pk + ctx
                        cnt_t = small.tile([P, 1], f32, tag="cnt")
                        nc.vector.tensor_scalar(out=cnt_t[:],
                                                in0=ps[ctx][:, D : D + 1],
                                                scalar1=1.0, scalar2=None,
                                                op0=OP.max)
                        rcp_t = small.tile([P, 1], f32, tag="rcp")
                        nc.vector.reciprocal(out=rcp_t[:], in_=cnt_t[:])
                        osl = o_t[:, wloc * D : (wloc + 1) * D]
                        nc.scalar.activation(
                            out=osl,
                            in_=ps[ctx][:, 0:D],
                            func=mybir.ActivationFunctionType.Copy,
                            scale=rcp_t[:, 0:1],
                        )
                        nc.vector.tensor_tensor(
                            out=osl, in0=osl,
                            in1=x_t[:, wloc * D : (wloc + 1) * D],
                            op=OP.add,
                        )

                nc.sync.dma_start(
                    out=out[roff : roff + RPC, :].rearrange(
                        "(p w) d -> p (w d)", p=P
                    ),
                    in_=o_t[:, : 2 * CP * D],
                )
                roff += RPC

    nc.compile()
    return nc


_P1_CACHE: list = []
_P2_CACHE: dict = {}


def _get_program1():
    if not _P1_CACHE:
        _P1_CACHE.append(build_program1())
    return _P1_CACHE[0]


def _get_program2(PC, CPs):
    key = (PC, tuple(CPs))
    if key not in _P2_CACHE:
        _P2_CACHE[key] = build_program2(PC, tuple(CPs))
    return _P2_CACHE[key]


# ---------------------------------------------------------------- host prep
def _prep_pass1(edge_time, seed_time, dst):
    et = np.asarray(edge_time).astype(np.float16)
    st_d = np.asarray(seed_time)[dst].astype(np.float16)
    etp = np.full(EPAD, 30000.0, dtype=np.float16)
    stp = np.zeros(EPAD, dtype=np.float16)
    etp[:E] = et
    stp[:E] = st_d
    etp = etp.reshape(NCORES, P, CE)
    stp = stp.reshape(NCORES, P, CE)
    return [{"etp": etp[c], "stp": stp[c]} for c in range(NCORES)]


def _decode_mask(res):
    m = np.stack([res.results[c]["msk"] for c in range(NCORES)])
    return m.reshape(-1)[:E] > 0.5


def _pack_pairs(cnt_db, active):
    """Snake-deal active dsts (sorted by total cnt desc) into window pairs,
    then fill to 256 dsts/pair with inactive fillers.  Returns
    (PC, pair_dsts [8*PC, 256] int64)."""
    tot = cnt_db.sum(1)
    act_ids = np.flatnonzero(active)
    A = act_ids.size
    PC = max(1, math.ceil(A / (NCORES * 256)))
    while True:
        npairs = NCORES * PC
        order = act_ids[np.argsort(-tot[act_ids], kind="stable")]
        # snake deal
        cols = np.arange(order.size)
        rnd = cols // npairs
        pos = cols % npairs
        pair_of = np.where(rnd % 2 == 0, pos, npairs - 1 - pos)
        # per-pair per-bank loads
        loads = np.zeros((npairs, NBANKS), dtype=np.int64)
        np.add.at(loads, pair_of, cnt_db[order])
        counts = np.bincount(pair_of, minlength=npairs)
        if loads.max() <= P and counts.max() <= 256:
            break
        PC += 1
    # fill with inactive dsts
    pair_dsts = np.full((npairs, 256), -1, dtype=np.int64)
    perm = np.argsort(pair_of, kind="stable")
    sor = order[perm]
    po = pair_of[perm]
    # position within pair
    first = np.r_[True, po[1:] != po[:-1]]
    idx0 = np.flatnonzero(first)
    runlen = np.diff(np.r_[idx0, po.size])
    within = np.arange(po.size) - np.repeat(idx0, runlen)
    pair_dsts[po, within] = sor
    # fillers: inactive dsts
    inact = np.flatnonzero(~active)
    need = (pair_dsts < 0)
    nneed = need.sum()
    fill = np.full(nneed, -1, dtype=np.int64)
    take = min(nneed, inact.size)
    fill[:take] = inact[:take]
    pair_dsts[need] = fill
    used_inact = inact[:take]
    return PC, pair_dsts, used_inact


def _prep_pass2(x, src, dst, mask):
    x = np.asarray(x, dtype=np.float32)
    me = np.flatnonzero(mask)
    src_m = src[me]
    dst_m = dst[me]
    bank_m = src_m // BANK
    cnt_db = np.zeros((N, NBANKS), dtype=np.int64)
    np.add.at(cnt_db, (dst_m, bank_m), 1)
    active = cnt_db.sum(1) > 0
    PC, pair_dsts, used_inact = _pack_pairs(cnt_db, active)
    npairs = NCORES * PC
    NCH = min(2, PC)
    base = PC // NCH
    rem = PC % NCH
    CPs = tuple(base + (1 if i < rem else 0) for i in range(NCH))

    # dst -> (core-local slot) mapping
    dloc = np.full(N, -1, dtype=np.int64)       # core-local dst index
    dcore = np.full(N, -1, dtype=np.int64)
    pd = pair_dsts.reshape(NCORES, PC, 256)
    for c in range(NCORES):
        ids = pd[c]
        v = ids >= 0
        dloc[ids[v]] = (np.arange(PC * 256).reshape(PC, 256))[v]
        dcore[ids[v]] = c

    # masked edge placement: (core, pair-local pc, bank, slot)
    ecore = dcore[dst_m]
    epair = dloc[dst_m] // 256                  # pair index within core
    # rank within (core, pair, bank)
    key = ((ecore * PC + epair) * NBANKS + bank_m).astype(np.int64)
    eorder = np.argsort(key, kind="stable")
    ks = key[eorder]
    binc = np.bincount(key, minlength=NCORES * PC * NBANKS)
    assert binc.max() <= P
    offs = np.zeros(NCORES * PC * NBANKS, dtype=np.int64)
    np.cumsum(binc[:-1], out=offs[1:])
    rank = np.arange(me.size, dtype=np.int64) - offs[ks]

    cso = ecore[eorder]
    pso = epair[eorder]
    jso = bank_m[eorder]
    srcso = src_m[eorder]
    dlocso = dloc[dst_m][eorder]

    # chunk bookkeeping
    chunk_of_pair = np.zeros(PC, dtype=np.int64)
    pk_in_chunk = np.zeros(PC, dtype=np.int64)
    cstart = 0
    ioffs = []      # idx col offset per (chunk)
    boffs = []
    ioff = 0
    boff = 0
    for ci, CP in enumerate(CPs):
        chunk_of_pair[cstart : cstart + CP] = ci
        pk_in_chunk[cstart : cstart + CP] = np.arange(CP)
        ioffs.append(ioff)
        boffs.append(boff)
        ioff += NBANKS * CP * P // 16
        boff += NBANKS * CP
        cstart += CP
    TOTI = ioff

    co = chunk_of_pair[pso]
    pko = pk_in_chunk[pso]
    CPs_a = np.asarray(CPs)
    ioffs_a = np.asarray(ioffs)
    boffs_a = np.asarray(boffs)

    # gather idx planes: call (chunk, bank j): position i = pk*128 + rank
    pos = pko * P + rank
    icol = ioffs_a[co] + jso * (CPs_a[co] * P // 16) + pos // 16
    irow = pos % 16
    idx_a = np.zeros((NCORES, 16, TOTI), dtype=np.int16)
    idx_a[cso, irow, icol] = (srcso - jso * BANK).astype(np.int16)
    idx_rep = np.tile(idx_a, (1, 8, 1))

    # dl plane: col = boff_c + j*CP + pk ; value = dloc - pc*256
    dl_a = np.full((NCORES, P, NBANKS * PC), 30000.0, dtype=np.float16)
    dcol = boffs_a[co] + jso * CPs_a[co] + pko
    dl_a[cso, rank, dcol] = (dlocso - pso * 256).astype(np.float16)

    # xs rows, permuted per chunk: row = roff_c + p*(2*CP) + wloc
    xg = np.zeros((NCORES, PC * 256, D), dtype=np.float32)
    for c in range(NCORES):
        ids = pd[c].reshape(-1)
        v = ids >= 0
        xg[c][v] = x[ids[v]]
    xs_l = []
    for c in range(NCORES):
        rows = []
        rbase = 0
        for ci, CP in enumerate(CPs):
            blk = xg[c][rbase : rbase + CP * 256]     # (wloc p) d  w/ dloc = wloc*128+p
            blk = blk.reshape(2 * CP, P, D).transpose(1, 0, 2).reshape(-1, D)
            rows.append(blk)
            rbase += CP * 256
        xs_l.append(np.ascontiguousarray(np.concatenate(rows, 0)))

    x16 = np.zeros((XROWS, 2 * D), dtype=np.float16)
    x16[:N, :D] = x.astype(np.float16)
    x16[:, D] = 1.0

    in_maps = [
        {
            "x16": x16,
            "xs": xs_l[c],
            "idx16": idx_rep[c],
            "dlp": dl_a[c],
        }
        for c in range(NCORES)
    ]
    meta = {"PC": PC, "CPs": CPs, "pd": pd, "used_inact": used_inact}
    return in_maps, meta


def _assemble(x, res, meta):
    PC, CPs, pd = meta["PC"], meta["CPs"], meta["pd"]
    out = np.asarray(x, dtype=np.float32).copy()
    for c in range(NCORES):
        o = res.results[c]["out"]
        rbase = 0
        rows = []
        for CP in CPs:
            blk = o[rbase : rbase + CP * 256]
            blk = blk.reshape(P, 2 * CP, D).transpose(1, 0, 2).reshape(-1, D)
            rows.append(blk)
            rbase += CP * 256
        o_un = np.concatenate(rows, 0)           # dloc-ordered
        ids = pd[c].reshape(-1)
        v = ids >= 0
        out[ids[v]] = o_un[v]
    return out


def kernel(x, edge_index, edge_time, seed_time):
    ei = np.asarray(edge_index)
    src = ei[0].astype(np.int64)
    dst = ei[1].astype(np.int64)

    p1_maps = _prep_pass1(edge_time, seed_time, dst)
    nc1 = _get_program1()
    res1 = run_bass_kernel_spmd(nc1, p1_maps, core_ids=list(range(NCORES)))
    mask = _decode_mask(res1)

    p2_maps, meta = _prep_pass2(x, src, dst, mask)
    nc2 = _get_program2(meta["PC"], meta["CPs"])
    res2 = run_bass_kernel_spmd(nc2, p2_maps, core_ids=list(range(NCORES)))
    return _assemble(x, res2, meta).astype(np.float32)


# revision 27
# speedup vs baseline: 1.5070x; 1.2197x over previous
"""Trainium2 Bass kernel for temporal-window GNN mean aggregation.

    out = x + scatter_mean(x[src] * mask, dst),
    mask = (edge_time <= seed_time[dst]) & (edge_time > seed_time[dst] - 100)

Two-pass design; all reference arithmetic happens on device, host work is
layout only (permutation / compaction / padding by device-computed values).

Pass 1 (device): DVE computes the temporal mask for all E edges from
  metadata planes (edge_time, seed_time[dst]) laid out by the host.
Host: reads the mask back and uses it purely for layout: keeps only masked
  edges (~10%), packs the active destination nodes (cnt>=1) into balanced
  256-dst "window pairs", shards pairs across the 8 cores, and builds
  int16 gather-index planes + one-hot metadata for pass 2.  Inactive
  destinations get out = x + 0 = x, copied by the host.
Pass 2 (device): per core, 4 banked SWDGE dma_gathers fetch the masked
  x16 rows (512B = 128 fp16 features + 1.0 ones column); DVE builds the
  one-hot S from dst metadata; PE accumulates PSUM[dst 0:129] via
  S^T @ G matmuls (2 windows per 128-slot block, 4 banks); counts ride
  the ones column; ACT applies 1/max(cnt,1); DVE adds the residual.
"""

import math
import sys

import numpy as np

for _p in ("/opt/trn_rl_repo",):
    if _p not in sys.path:
        sys.path.insert(0, _p)

import concourse.bass as bass
import concourse.mybir as mybir
import concourse.tile as tile
from concourse import bacc
from concourse.bass_utils import run_bass_kernel_spmd

P = 128            # SBUF partitions == dst-window size == slot-block size
D = 128            # feature dim
NCORES = 8
N = 100000
E = 600000
TIME_WINDOW = 100

NBANKS = 4         # int16 gather-index banks over x16 rows
BANK = 25089       # rows per bank (<= 32768), NBANKS*BANK >= N
XROWS = NBANKS * BANK

# pass-1 edge grid
CE = 586                       # cols per partition; 128*CE = 75008 >= E/8
EPC = P * CE                   # edges per core (padded)
EPAD = NCORES * EPC

f32 = mybir.dt.float32
f16 = mybir.dt.float16
i32 = mybir.dt.int32
i16 = mybir.dt.int16
OP = mybir.AluOpType


# ---------------------------------------------------------------- pass 1
def build_program1():
    nc = bacc.Bacc("TRN2", target_bir_lowering=False, debug=False,
                   num_devices=NCORES)
    etp = nc.dram_tensor("etp", [P, CE], f16, kind="ExternalInput")
    stp = nc.dram_tensor("stp", [P, CE], f16, kind="ExternalInput")
    msk = nc.dram_tensor("msk", [P, CE], f16, kind="ExternalOutput")
    with tile.TileContext(nc) as tc:
        with tc.tile_pool(name="m", bufs=1) as pool:
            et_t = pool.tile([P, CE], f16)
            st_t = pool.tile([P, CE], f16)
            nc.sync.dma_start(out=et_t[:], in_=etp[:])
            nc.sync.dma_start(out=st_t[:], in_=stp[:])
            d_t = pool.tile([P, CE], f16)
            nc.vector.tensor_tensor(out=d_t[:], in0=st_t[:], in1=et_t[:],
                                    op=OP.subtract)
            m1_t = pool.tile([P, CE], f16)
            nc.vector.tensor_scalar(out=m1_t[:], in0=d_t[:], scalar1=0.0,
                                    scalar2=None, op0=OP.is_ge)
            m2_t = pool.tile([P, CE], f16)
            nc.vector.tensor_scalar(out=m2_t[:], in0=d_t[:],
                                    scalar1=float(TIME_WINDOW),
                                    scalar2=None, op0=OP.is_lt)
            m_t = pool.tile([P, CE], f16)
            nc.vector.tensor_tensor(out=m_t[:], in0=m1_t[:], in1=m2_t[:],
                                    op=OP.mult)
            nc.sync.dma_start(out=msk[:], in_=m_t[:])
    nc.compile()
    return nc


# ---------------------------------------------------------------- pass 2
def build_program2(PC: int, CPs: tuple):
    """PC pairs of 128-dst windows per core, chunked into CPs pairs/chunk.

    Per (chunk, bank): one dma_gather of CP*128 slots; block b holds pair
    pk's bank-j masked edges (<=128 by host packing).  Each block feeds two
    matmuls (window contexts 2*pk and 2*pk+1) via iota2-vs-dl one-hot.
    """
    assert sum(CPs) == PC
    CPmax = max(CPs)
    nc = bacc.Bacc(
        "TRN2", target_bir_lowering=False, debug=False, num_devices=NCORES,
        num_swdge_queues=4,
    )

    TOTI = PC * NBANKS * P // 16          # total idx cols (int16, wrapped)
    x16 = nc.dram_tensor("x16", [XROWS, 2 * D], f16, kind="ExternalInput")
    xs = nc.dram_tensor("xs", [PC * 2 * P, D], f32, kind="ExternalInput")
    idx16 = nc.dram_tensor("idx16", [P, TOTI], i16, kind="ExternalInput")
    dlp = nc.dram_tensor("dlp", [P, NBANKS * PC], f16, kind="ExternalInput")
    out = nc.dram_tensor("out", [PC * 2 * P, D], f32, kind="ExternalOutput")

    with tile.TileContext(nc) as tc:
        with (
            tc.tile_pool(name="meta", bufs=1) as meta,
            tc.tile_pool(name="oc", bufs=2) as oc,
            tc.tile_pool(name="small", bufs=4) as small,
            tc.tile_pool(name="psum", bufs=4, space="PSUM") as psum_tp,
        ):
            # lean preamble: ONLY the idx load gates the gathers.  dl/iota/
            # xs loads are needed no earlier than the S-build / finalize and
            # are issued after chunk-0's gathers are on the queue.
            dl_t = meta.tile([P, NBANKS * PC], f16)
            idx_t = meta.tile([P, TOTI], i16)
            nc.sync.dma_start(out=idx_t[:], in_=idx16[:])

            iota_i = meta.tile([P, 2 * P], i32)
            iota_f = meta.tile([P, 2 * P], f16)

            # hoisted residual-x reads: issued up front so this HWDGE
            # traffic drains before the gather transfers stream.
            x_ts = []
            roff0 = 0
            for c, CP in enumerate(CPs):
                RPC0 = 2 * CP * P
                x_t = meta.tile([P, 2 * CP * D], f32, name=f"x_{c}")
                nc.sync.dma_start(
                    out=x_t[:],
                    in_=xs[roff0 : roff0 + RPC0, :].rearrange(
                        "(p w) d -> p (w d)", p=P
                    ),
                )
                x_ts.append(x_t)
                roff0 += RPC0

            ioff = 0
            boff = 0
            roff = 0
            for c, CP in enumerate(CPs):
                NIDX = CP * P
                ICOLS = NIDX // 16
                g_ts = []
                for j in range(NBANKS):
                    g_t = meta.tile([P, CP * 2 * D], f16,
                                    name=f"g_{c}_{j}")
                    nc.gpsimd.dma_gather(
                        out_ap=g_t[:].rearrange("p (k c) -> p k c", c=2 * D),
                        in_ap=x16[j * BANK :, :],
                        idxs_ap=idx_t[:, ioff + j * ICOLS
                                      : ioff + (j + 1) * ICOLS],
                        num_idxs=NIDX,
                        num_idxs_reg=NIDX,
                        elem_size=2 * D,
                        single_packet=False,
                        queue_num=j,
                    )
                    g_ts.append(g_t)
                ioff += NBANKS * ICOLS

                if c == 0:
                    # deferred: dl (S-build input) and the iota ramp are not
                    # needed before the matmul phase.
                    nc.sync.dma_start(out=dl_t[:], in_=dlp[:])
                    nc.gpsimd.iota(iota_i[:], pattern=[[0, 1], [1, 2 * P]],
                                   base=0, channel_multiplier=0)
                    nc.vector.tensor_copy(out=iota_f[:], in_=iota_i[:])

                BC = NBANKS * CP
                s_t = meta.tile([P, BC * 2 * P], f16, name=f"s_{c}")
                nc.vector.tensor_tensor(
                    out=s_t[:].rearrange("p (b m) -> p b m", m=2 * P),
                    in0=iota_f[:].unsqueeze(1).to_broadcast([P, BC, 2 * P]),
                    in1=dl_t[:, boff : boff + BC]
                    .unsqueeze(2)
                    .to_broadcast([P, BC, 2 * P]),
                    op=OP.is_equal,
                )
                boff += BC

                RPC = 2 * CP * P
                x_t = x_ts[c]

                o_t = oc.tile([P, 2 * CPmax * D], f32, tag="o")
                for pk in range(CP):
                    ps = [psum_tp.tile([P, D + 1], f32, tag=f"ps{ctx}",
                                       name=f"ps{ctx}_{c}_{pk}")
                          for ctx in range(2)]
                    for j in range(NBANKS):
                        b = j * CP + pk
                        for ctx in range(2):
                            nc.tensor.matmul(
                                out=ps[ctx][:],
                                lhsT=s_t[:, b * 2 * P + ctx * P
                                         : b * 2 * P + (ctx + 1) * P],
                                rhs=g_ts[j][:, pk * 2 * D : pk * 2 * D + D + 1],
                                start=(j == 0),
                                stop=(j == NBANKS - 1),
                            )
                    for ctx in range(2):
                        wloc = 2 * pk + ctx
                        cnt_t = small.tile([P, 1], f32, tag="cnt")
                        nc.vector.tensor_scalar(out=cnt_t[:],
                                                in0=ps[ctx][:, D : D + 1],
                                                scalar1=1.0, scalar2=None,
                                                op0=OP.max)
                        rcp_t = small.tile([P, 1], f32, tag="rcp")
                        nc.vector.reciprocal(out=rcp_t[:], in_=cnt_t[:])
                        osl = o_t[:, wloc * D : (wloc + 1) * D]
                        nc.scalar.activation(
                            out=osl,
                            in_=ps[ctx][:, 0:D],
                            func=mybir.ActivationFunctionType.Copy,
                            scale=rcp_t[:, 0:1],
                        )
                        nc.vector.tensor_tensor(
                            out=osl, in0=osl,
                            in1=x_t[:, wloc * D : (wloc + 1) * D],
                            op=OP.add,
                        )

                nc.sync.dma_start(
                    out=out[roff : roff + RPC, :].rearrange(
                        "(p w) d -> p (w d)", p=P
                    ),
                    in_=o_t[:, : 2 * CP * D],
                )
                roff += RPC

    nc.compile()
    return nc


_P1_CACHE: list = []
_P2_CACHE: dict = {}


def _get_program1():
    if not _P1_CACHE:
        _P1_CACHE.append(build_program1())
    return _P1_CACHE[0]


def _get_program2(PC, CPs):
    key = (PC, tuple(CPs))
    if key not in _P2_CACHE:
        _P2_CACHE[key] = build_program2(PC, tuple(CPs))
    return _P2_CACHE[key]


# ---------------------------------------------------------------- host prep
def _prep_pass1(edge_time, seed_time, dst):
    et = np.asarray(edge_time).astype(np.float16)
    st_d = np.asarray(seed_time)[dst].astype(np.float16)
    etp = np.full(EPAD, 30000.0, dtype=np.float16)
    stp = np.zeros(EPAD, dtype=np.float16)
    etp[:E] = et
    stp[:E] = st_d
    etp = etp.reshape(NCORES, P, CE)
    stp = stp.reshape(NCORES, P, CE)
    return [{"etp": etp[c], "stp": stp[c]} for c in range(NCORES)]


def _decode_mask(res):
    m = np.stack([res.results[c]["msk"] for c in range(NCORES)])
    return m.reshape(-1)[:E] > 0.5


def _pack_pairs(cnt_db, active):
    """Snake-deal active dsts (sorted by total cnt desc) into window pairs,
    then fill to 256 dsts/pair with inactive fillers.  Returns
    (PC, pair_dsts [8*PC, 256] int64)."""
    tot = cnt_db.sum(1)
    act_ids = np.flatnonzero(active)
    A = act_ids.size
    PC = max(1, math.ceil(A / (NCORES * 256)))
    while True:
        npairs = NCORES * PC
        order = act_ids[np.argsort(-tot[act_ids], kind="stable")]
        # snake deal
        cols = np.arange(order.size)
        rnd = cols // npairs
        pos = cols % npairs
        pair_of = np.where(rnd % 2 == 0, pos, npairs - 1 - pos)
        # per-pair per-bank loads
        loads = np.zeros((npairs, NBANKS), dtype=np.int64)
        np.add.at(loads, pair_of, cnt_db[order])
        counts = np.bincount(pair_of, minlength=npairs)
        if loads.max() <= P and counts.max() <= 256:
            break
        PC += 1
    # fill with inactive dsts
    pair_dsts = np.full((npairs, 256), -1, dtype=np.int64)
    perm = np.argsort(pair_of, kind="stable")
    sor = order[perm]
    po = pair_of[perm]
    # position within pair
    first = np.r_[True, po[1:] != po[:-1]]
    idx0 = np.flatnonzero(first)
    runlen = np.diff(np.r_[idx0, po.size])
    within = np.arange(po.size) - np.repeat(idx0, runlen)
    pair_dsts[po, within] = sor
    # fillers: inactive dsts
    inact = np.flatnonzero(~active)
    need = (pair_dsts < 0)
    nneed = need.sum()
    fill = np.full(nneed, -1, dtype=np.int64)
    take = min(nneed, inact.size)
    fill[:take] = inact[:take]
    pair_dsts[need] = fill
    used_inact = inact[:take]
    return PC, pair_dsts, used_inact


def _prep_pass2(x, src, dst, mask):
    x = np.asarray(x, dtype=np.float32)
    me = np.flatnonzero(mask)
    src_m = src[me]
    dst_m = dst[me]
    bank_m = src_m // BANK
    cnt_db = np.zeros((N, NBANKS), dtype=np.int64)
    np.add.at(cnt_db, (dst_m, bank_m), 1)
    active = cnt_db.sum(1) > 0
    PC, pair_dsts, used_inact = _pack_pairs(cnt_db, active)
    npairs = NCORES * PC
    if PC >= 6:
        # tiny first chunk: the 4 queues' first gather calls are short, so
        # every queue's transfer stream (and the first matmuls) start early
        # instead of serializing behind one long engine-held descgen.
        rest = PC - 2
        CPs = (2, (rest + 1) // 2, rest // 2)
    else:
        NCH = min(2, PC)
        base = PC // NCH
        rem = PC % NCH
        CPs = tuple(base + (1 if i < rem else 0) for i in range(NCH))

    # dst -> (core-local slot) mapping
    dloc = np.full(N, -1, dtype=np.int64)       # core-local dst index
    dcore = np.full(N, -1, dtype=np.int64)
    pd = pair_dsts.reshape(NCORES, PC, 256)
    for c in range(NCORES):
        ids = pd[c]
        v = ids >= 0
        dloc[ids[v]] = (np.arange(PC * 256).reshape(PC, 256))[v]
        dcore[ids[v]] = c

    # masked edge placement: (core, pair-local pc, bank, slot)
    ecore = dcore[dst_m]
    epair = dloc[dst_m] // 256                  # pair index within core
    # rank within (core, pair, bank)
    key = ((ecore * PC + epair) * NBANKS + bank_m).astype(np.int64)
    eorder = np.argsort(key, kind="stable")
    ks = key[eorder]
    binc = np.bincount(key, minlength=NCORES * PC * NBANKS)
    assert binc.max() <= P
    offs = np.zeros(NCORES * PC * NBANKS, dtype=np.int64)
    np.cumsum(binc[:-1], out=offs[1:])
    rank = np.arange(me.size, dtype=np.int64) - offs[ks]

    cso = ecore[eorder]
    pso = epair[eorder]
    jso = bank_m[eorder]
    srcso = src_m[eorder]
    dlocso = dloc[dst_m][eorder]

    # chunk bookkeeping
    chunk_of_pair = np.zeros(PC, dtype=np.int64)
    pk_in_chunk = np.zeros(PC, dtype=np.int64)
    cstart = 0
    ioffs = []      # idx col offset per (chunk)
    boffs = []
    ioff = 0
    boff = 0
    for ci, CP in enumerate(CPs):
        chunk_of_pair[cstart : cstart + CP] = ci
        pk_in_chunk[cstart : cstart + CP] = np.arange(CP)
        ioffs.append(ioff)
        boffs.append(boff)
        ioff += NBANKS * CP * P // 16
        boff += NBANKS * CP
        cstart += CP
    TOTI = ioff

    co = chunk_of_pair[pso]
    pko = pk_in_chunk[pso]
    CPs_a = np.asarray(CPs)
    ioffs_a = np.asarray(ioffs)
    boffs_a = np.asarray(boffs)

    # gather idx planes: call (chunk, bank j): position i = pk*128 + rank
    pos = pko * P + rank
    icol = ioffs_a[co] + jso * (CPs_a[co] * P // 16) + pos // 16
    irow = pos % 16
    idx_a = np.zeros((NCORES, 16, TOTI), dtype=np.int16)
    idx_a[cso, irow, icol] = (srcso - jso * BANK).astype(np.int16)
    idx_rep = np.tile(idx_a, (1, 8, 1))

    # dl plane: col = boff_c + j*CP + pk ; value = dloc - pc*256
    dl_a = np.full((NCORES, P, NBANKS * PC), 30000.0, dtype=np.float16)
    dcol = boffs_a[co] + jso * CPs_a[co] + pko
    dl_a[cso, rank, dcol] = (dlocso - pso * 256).astype(np.float16)

    # xs rows, permuted per chunk: row = roff_c + p*(2*CP) + wloc
    xg = np.zeros((NCORES, PC * 256, D), dtype=np.float32)
    for c in range(NCORES):
        ids = pd[c].reshape(-1)
        v = ids >= 0
        xg[c][v] = x[ids[v]]
    xs_l = []
    for c in range(NCORES):
        rows = []
        rbase = 0
        for ci, CP in enumerate(CPs):
            blk = xg[c][rbase : rbase + CP * 256]     # (wloc p) d  w/ dloc = wloc*128+p
            blk = blk.reshape(2 * CP, P, D).transpose(1, 0, 2).reshape(-1, D)
            rows.append(blk)
            rbase += CP * 256
        xs_l.append(np.ascontiguousarray(np.concatenate(rows, 0)))

    x16 = np.zeros((XROWS, 2 * D), dtype=np.float16)
    x16[:N, :D] = x.astype(np.float16)
    x16[:, D] = 1.0

    in_maps = [
        {
            "x16": x16,
            "xs": xs_l[c],
            "idx16": idx_rep[c],
            "dlp": dl_a[c],
        }
        for c in range(NCORES)
    ]
    meta = {"PC": PC, "CPs": CPs, "pd": pd, "used_inact": used_inact}
    return in_maps, meta


def _assemble(x, res, meta):
    PC, CPs, pd = meta["PC"], meta["CPs"], meta["pd"]
    out = np.asarray(x, dtype=np.float32).copy()
    for c in range(NCORES):
        o = res.results[c]["out"]
        rbase = 0
        rows = []
        for CP in CPs:
            blk = o[rbase : rbase + CP * 256]
            blk = blk.reshape(P, 2 * CP, D).transpose(1, 0, 2).reshape(-1, D)
            rows.append(blk)
            rbase += CP * 256
        o_un = np.concatenate(rows, 0)           # dloc-ordered
        ids = pd[c].reshape(-1)
        v = ids >= 0
        out[ids[v]] = o_un[v]
    return out


def kernel(x, edge_index, edge_time, seed_time):
    ei = np.asarray(edge_index)
    src = ei[0].astype(np.int64)
    dst = ei[1].astype(np.int64)

    p1_maps = _prep_pass1(edge_time, seed_time, dst)
    nc1 = _get_program1()
    res1 = run_bass_kernel_spmd(nc1, p1_maps, core_ids=list(range(NCORES)))
    mask = _decode_mask(res1)

    p2_maps, meta = _prep_pass2(x, src, dst, mask)
    nc2 = _get_program2(meta["PC"], meta["CPs"])
    res2 = run_bass_kernel_spmd(nc2, p2_maps, core_ids=list(range(NCORES)))
    return _assemble(x, res2, meta).astype(np.float32)
